# revision 9
# baseline (speedup 1.0000x reference)
"""Trainium2 Bass kernel for batched 3-D k-NN local-covariance trace.

Problem: pcd [B=8, N=4096, 3] -> per-point trace of the 3x3 covariance of its
k=5 nearest neighbors (self included), normalized by the per-batch max.

Sharding: data-parallel over batch — core b owns batch b (N=4096 points).

Per-core algorithm (all SBUF-resident after the initial load):
  * rank value r[i,j] = 2*x_i.y_j - |y_j|^2 computed as a bf16 matmul with an
    error-free hi/lo split (hi*hi + hi*lo + lo*hi), scaled by S=2^22, PLUS
    index-packing rows that make the PSUM value itself carry the candidate
    index (this removes the full-row FIND_INDEX8 pass entirely — the old
    bottleneck, ~170us of DVE time):
      row 11: per-query bias  bf16(OFF + S*(C - |x_i|^2))  -> top values land
              in (0, 2^24), independent of |x_i|
      row 12: +M, row 13: -M with M = 3*2^31.  fp32 PSUM accumulation is
              sequential in k, so +M rounds the running sum to a multiple of
              512 (ULP at 3*2^31 exponent) and -M restores it exactly
              (Sterbenz) — a free "round to 512" inside the matmul
      rows 14/15: jhi + jlo = the column's chunk-local index (0..511), which
              lands exactly in the 9 low bits freed by the rounding
    Net: psum = round512(S*(C' - d2)) + j, monotone in -d2 up to a 1.2e-4
    quantization (ties swap near-equal neighbors only; validated rel err
    ~4.5e-3 vs the fp32 reference).
  * selection per 128-query row block: one DVE MAX8 per 512-wide PSUM bank
    (top-8 per chunk, read directly from PSUM — no scalar copies), an 8x8
    merge MAX8 + MAX_INDEX on the [128,64] concat, then a tiny decode:
    j = int(packed) & 511, chunk = slot >> 3, global = chunk*512 + j.
  * setup avoids 4-byte-strided pcd loads: pcd is DMA'd once contiguously as
    [128, 96], hi/lo split runs elementwise on all 128 lanes, and the [*, N]
    matmul rows / gather table are produced by DVE 32x32 block transposes +
    contiguous flatten DMAs.  This stores points in a fixed permutation
    n' = 32p + j; every tensor uses the same permutation, so only the final
    output DMA needs to invert it.
  * neighbor coordinate gather via gpsimd indirect_copy (per-16-partition
    group, coords on partitions 16g..16g+2 of the table).
  * stable centered trace per query; components summed across partitions with
    a tiny matmul against a constant selection matrix E.
  * global max over the 4096 traces (gpsimd partition_all_reduce) -> scale by
    1/(max+1e-8) -> multi-queue DMA out.
"""

import numpy as np
from contextlib import ExitStack

N = 4096
KNN = 5
P = 128          # queries per row block
NBLK = N // P    # 32 row blocks
CH = 512         # candidate chunk (one fp32 PSUM bank)
NCH = N // CH    # 8 chunks
G16 = 16         # partitions per gpsimd core group
NG = P // G16    # 8 groups per row block
KA = 16          # augmented contraction rows (hi/lo split + pack rows)

SCL = float(2 ** 22)       # value scale (power of two: exact bf16 scaling)
CC = 1.5                   # clamp center: covers d5 up to CC + 2^23/SCL = 3.5
OFF = float(2 ** 23)       # positivity offset
MAGIC = float(3 * 2 ** 31)  # +MAGIC then -MAGIC rounds PSUM to multiples of 512


def build_nc():
    import concourse.bass as bass
    import concourse.tile as tile
    from concourse import bacc, mybir
    from concourse import bass_isa

    dt = mybir.dt
    f32 = dt.float32
    bf16 = dt.bfloat16
    Alu = mybir.AluOpType
    Axis = mybir.AxisListType

    nc = bacc.Bacc("TRN2", target_bir_lowering=False, debug=False)
    pcd_d = nc.dram_tensor("pcd", [N, 3], f32, kind="ExternalInput")
    out_d = nc.dram_tensor("out", [N], f32, kind="ExternalOutput")

    with tile.TileContext(nc) as tc, ExitStack() as ctx:
        const = ctx.enter_context(tc.tile_pool(name="const", bufs=1))
        stage = ctx.enter_context(tc.tile_pool(name="stage", bufs=1))
        small = ctx.enter_context(tc.tile_pool(name="small", bufs=3))
        psum = ctx.enter_context(tc.tile_pool(name="psum", bufs=7, space="PSUM"))
        psacc = ctx.enter_context(tc.tile_pool(name="psacc", bufs=1, space="PSUM"))

        # ---- one-time setup -------------------------------------------------
        # contiguous load: partition p holds points 32p..32p+31 (96 floats)
        pw = stage.tile([P, 3 * 32], f32)
        nc.sync.dma_start(pw[:], pcd_d.ap().rearrange("(p j) d -> p (j d)", p=P))

        # hi/lo split, elementwise in the wide layout (all 128 lanes)
        ph = stage.tile([P, 3 * 32], bf16)       # bf16(x)       (lhsT rows)
        nc.scalar.copy(ph[:], pw[:])
        p2h = stage.tile([P, 3 * 32], bf16)      # bf16(2*S*x)   (rhs rows)
        nc.scalar.mul(p2h[:], pw[:], 2.0 * SCL)
        phf = stage.tile([P, 3 * 32], f32)
        nc.vector.tensor_copy(phf[:], ph[:])
        plo = stage.tile([P, 3 * 32], f32)       # x - fp32(bf16(x)), exact
        nc.vector.tensor_sub(plo[:], pw[:], phf[:])
        plob = stage.tile([P, 3 * 32], bf16)
        nc.scalar.copy(plob[:], plo[:])
        p2lob = stage.tile([P, 3 * 32], bf16)    # bf16(2*S*lo)
        nc.scalar.mul(p2lob[:], plo[:], 2.0 * SCL)

        # -|y|^2 per point, then its own hi/lo split in the scaled domain
        s3p = stage.tile([P, 3 * 32], f32)
        nc.vector.tensor_mul(s3p[:], pw[:], pw[:])
        sqn = stage.tile([P, 32], f32)           # -|y|^2 (fp32)
        nc.vector.tensor_reduce(sqn[:], s3p[:].rearrange("p (j d) -> p j d", d=3),
                                axis=Axis.X, op=Alu.add, negate=True)
        snh = stage.tile([P, 32], bf16)          # bf16(S*sqn)
        nc.scalar.mul(snh[:], sqn[:], SCL)
        snhf = stage.tile([P, 32], f32)
        nc.vector.tensor_copy(snhf[:], snh[:])
        sqnS = stage.tile([P, 32], f32)
        nc.vector.tensor_scalar_mul(sqnS[:], sqn[:], SCL)
        snlo = stage.tile([P, 32], f32)
        nc.vector.tensor_sub(snlo[:], sqnS[:], snhf[:])
        snlob = stage.tile([P, 32], bf16)
        nc.scalar.copy(snlob[:], snlo[:])

        # per-query bias row: bf16(OFF + S*CC + S*sqn) = bf16(OFF + S*(CC-|x|^2))
        bqv = stage.tile([P, 32], bf16)
        nc.scalar.activation(bqv[:], sqn[:], mybir.ActivationFunctionType.Copy,
                             scale=SCL, bias=OFF + SCL * CC)

        # chunk-local index rows: column n' = 32p + j -> n' mod 512
        ji = stage.tile([P, 32], dt.int16)
        nc.gpsimd.iota(ji[:], [[1, 32]], channel_multiplier=32)
        jcl = stage.tile([P, 32], dt.int16)
        nc.vector.tensor_scalar(jcl[:], ji[:], 511, None, op0=Alu.bitwise_and)
        jhi16 = stage.tile([P, 32], dt.int16)
        nc.vector.tensor_scalar(jhi16[:], jcl[:], 504, None, op0=Alu.bitwise_and)
        jlo16 = stage.tile([P, 32], dt.int16)
        nc.vector.tensor_scalar(jlo16[:], jcl[:], 7, None, op0=Alu.bitwise_and)
        jhib = stage.tile([P, 32], bf16)
        nc.vector.tensor_copy(jhib[:], jhi16[:])
        jlob = stage.tile([P, 32], bf16)
        nc.vector.tensor_copy(jlob[:], jlo16[:])

        # 32x32 block transposes; flattening (partition-major) then yields the
        # permuted point order n' shared by every [*, N] tensor below.
        def coord_view(t, d):
            return t[:].rearrange("p (j d) -> p d j", d=3)[:, d, :]

        tbl = const.tile([P, N], f32)        # gather table: coords on p%16<3
        xl = const.tile([KA, N], bf16)       # lhsT rows
        xr = const.tile([KA, N], bf16)       # rhs rows

        tp = [stage.tile([P, 32], f32, name=f"tp{d}") for d in range(3)]
        th = [stage.tile([P, 32], bf16, name=f"th{d}") for d in range(3)]
        t2h = [stage.tile([P, 32], bf16, name=f"t2h{d}") for d in range(3)]
        tlo = [stage.tile([P, 32], bf16, name=f"tlo{d}") for d in range(3)]
        t2lo = [stage.tile([P, 32], bf16, name=f"t2lo{d}") for d in range(3)]
        tsh = stage.tile([P, 32], bf16)
        tslo = stage.tile([P, 32], bf16)
        tbq = stage.tile([P, 32], bf16)
        for d in range(3):
            nc.vector.transpose(tp[d][:], coord_view(pw, d))
            nc.vector.transpose(th[d][:], coord_view(ph, d))
            nc.vector.transpose(t2h[d][:], coord_view(p2h, d))
            nc.vector.transpose(tlo[d][:], coord_view(plob, d))
            nc.vector.transpose(t2lo[d][:], coord_view(p2lob, d))
        nc.vector.transpose(tsh[:], snh[:])
        nc.vector.transpose(tslo[:], snlob[:])
        nc.vector.transpose(tbq[:], bqv[:])

        # constant rows (ones / +-MAGIC): gpsimd memsets must start at a
        # group-aligned partition, so memset the whole lhsT tile to ones
        # (value/bias rows are overwritten by the flatten DMAs below) and
        # stage the xr constant rows in partition-0 tiles.
        nc.gpsimd.memset(xl[:, :], 1.0)
        onesr = stage.tile([1, N], bf16)
        nc.gpsimd.memset(onesr[:], 1.0)
        magr = stage.tile([1, N], bf16)
        nc.gpsimd.memset(magr[:], MAGIC)
        nmagr = stage.tile([1, N], bf16)
        nc.gpsimd.memset(nmagr[:], -MAGIC)
        nc.sync.dma_start(xr[11:12, :], onesr[:])
        nc.scalar.dma_start(xr[12:13, :], magr[:])
        nc.gpsimd.dma_start(xr[13:14, :], nmagr[:])

        # flatten DMAs (contiguous 128B/64B runs) into the row tiles;
        # xr/xl first so the main loop's matmuls can start ASAP (DMA queues
        # drain in issue order), the gather table after
        qs = (nc.sync, nc.scalar, nc.gpsimd)
        for d in range(3):
            qs[d].dma_start(xr[d:d + 1, :], t2h[d][:])
            qs[(d + 1) % 3].dma_start(xr[3 + d:4 + d, :], t2lo[d][:])
            qs[(d + 2) % 3].dma_start(xl[d:d + 1, :], th[d][:])
            qs[d].dma_start(xl[6 + d:7 + d, :], tlo[d][:])
        for d in range(3):
            qs[(d + 1) % 3].dma_start(xl[3 + d:4 + d, :], th[d][:])
            qs[(d + 2) % 3].dma_start(xr[6 + d:7 + d, :], t2h[d][:])
        nc.sync.dma_start(xr[9:10, :], tsh[:])
        nc.scalar.dma_start(xr[10:11, :], tslo[:])
        nc.gpsimd.dma_start(xl[11:12, :], tbq[:])
        nc.sync.dma_start(xr[14:15, :], jhib[:])
        nc.scalar.dma_start(xr[15:16, :], jlob[:])
        nc.gpsimd.memset(tbl[:], 0.0)
        for d in range(3):
            qs[d].dma_start(tbl[d:d + 1, :], tp[d][:])

        # replicate coords to every 16-partition group of tbl
        engs = (nc.sync, nc.scalar, nc.gpsimd, nc.sync,
                nc.scalar, nc.gpsimd, nc.sync)
        for g in range(1, NG):
            engs[g - 1].dma_start(tbl[G16 * g:G16 * g + 3, :], tbl[0:3, :])

        # E[p, g] = 1 iff p//16 == g and p%16 < 3  (component-sum selector)
        ones3 = const.tile([3, 1], f32)
        nc.vector.memset(ones3[:], 1.0)
        esel = const.tile([P, NG], f32)
        nc.vector.memset(esel[:], 0.0)
        for j in range(NG):
            g = 2 * (j & 3) + (j >> 2)
            nc.sync.dma_start(esel[G16 * g:G16 * g + 3, j:j + 1], ones3[:])

        warm = stage.tile([G16, 1], f32)
        nc.vector.memset(warm[:], 0.0)
        warm2 = stage.tile([G16, 1], f32)
        nc.gpsimd.partition_all_reduce(warm2[:], warm[:], channels=G16,
                                       reduce_op=bass_isa.ReduceOp.max)

        tr_sb = const.tile([G16, NG * NBLK], f32)
        # free layout of tr_sb: f = 64b + 32gl + 4rb + gh for row block
        # r = 8b + rb and group g = 2gh + gl  ->  DRAM block b is contiguous
        tr_view = tr_sb[:].rearrange("q (b gl rb gh) -> q b gl rb gh",
                                     b=4, gl=2, rb=8, gh=4)

        # ---- main loop over row blocks (4-stage software pipeline) ---------
        # stage A1(r):  matmuls -> per-chunk MAX8 into v64
        # stage A2(r-1): merge/max_index -> decode -> gather issue
        # stage B(r-2): ssum -> mean -> centered squares (gpsimd)
        # stage C(r-3): tt reduce -> component-sum matmul -> tr store
        # Tail stages run behind so the strict-FIFO DVE queue never
        # head-of-line blocks the next block's MAX8 scans on cross-engine
        # latency (gather on gpsimd, mean on scalar).
        def stage_a2(st):
            r, v64 = st["r"], st["v64"]
            vm8 = small.tile([P, 8], f32, tag="vm8")
            nc.vector.max(vm8[:], v64[:])
            slot8 = small.tile([P, 8], dt.uint16, tag="slot8")
            nc.vector.max_index(slot8[:], vm8[:], v64[:])
            # decode: j = int(packed) & 511 ; chunk = slot>>3 ; g = chunk*512+j
            pki = small.tile([P, 8], dt.int32, tag="pki")
            nc.vector.tensor_copy(pki[:], vm8[:])
            jt = small.tile([P, 8], dt.int32, tag="jt")
            nc.vector.tensor_scalar(jt[:], pki[:], 511, None, op0=Alu.bitwise_and)
            jt16 = small.tile([P, 8], dt.uint16, tag="jt16")
            nc.vector.tensor_copy(jt16[:], jt[:])
            cb = small.tile([P, 8], dt.uint16, tag="cb")
            nc.vector.tensor_scalar(cb[:], slot8[:], 3, 9,
                                    op0=Alu.logical_shift_right,
                                    op1=Alu.logical_shift_left)
            gidx = small.tile([P, 8], dt.uint16, tag="gidx")
            nc.vector.tensor_add(gidx[:], cb[:], jt16[:])
            # gather: group g gathers, for its 16 queries, slot-major:
            # gath[p, s*16+q16] = tbl[p, gidx[16*(p//16)+q16, s]]
            gath = small.tile([P, KNN * G16], f32, tag="gath")
            nc.gpsimd.indirect_copy(gath[:], tbl[:], gidx[:, :KNN], True)
            return {"gath": gath, "r": r}

        def stage_b(st):
            gv = st["gath"][:].rearrange("p (s q) -> p q s", s=KNN, q=G16)
            ssum = small.tile([P, G16], f32, tag="ssum")
            nc.vector.tensor_reduce(ssum[:], gv, axis=Axis.X, op=Alu.add)
            mean = small.tile([P, G16], f32, tag="mean")
            nc.scalar.mul(mean[:], ssum[:], 1.0 / KNN)
            cent = small.tile([P, G16, KNN], f32, tag="cent")
            nc.gpsimd.tensor_sub(cent[:], gv,
                                 mean[:].unsqueeze(2).broadcast_to([P, G16, KNN]))
            nc.gpsimd.tensor_mul(cent[:], cent[:], cent[:])
            return {"cent": cent, "r": st["r"]}

        def stage_c(st):
            r = st["r"]
            tt = small.tile([P, G16], f32, tag="tt")
            nc.vector.tensor_reduce(tt[:], st["cent"][:], axis=Axis.X, op=Alu.add)
            ps_tr = psacc.tile([G16, NG], f32, tag="tr")
            nc.tensor.matmul(ps_tr[:], tt[:], esel[:], start=True, stop=True)
            nc.scalar.copy(tr_view[:, r // 8, :, r % 8, :],
                           ps_tr[:].rearrange("q (gl gh) -> q gl gh", gl=2))

        stA2 = stB = stC = None
        for r in range(NBLK):
            lhsT = xl[:, r * P:(r + 1) * P]
            v64 = small.tile([P, NCH * 8], f32, tag="v64")
            for c in range(NCH):
                sl = slice(c * CH, (c + 1) * CH)
                ps = psum.tile([P, CH], f32, tag="mm")
                nc.tensor.matmul(ps[:], lhsT, xr[:, sl], start=True, stop=True)
                # top-8 of this chunk, straight from the PSUM bank
                nc.vector.max(v64[:, 8 * c:8 * c + 8], ps[:])

            newG = stage_a2(stA2) if stA2 else None
            if stC:
                stage_c(stC)
            stC = stage_b(stB) if stB else None
            stB = newG
            stA2 = {"v64": v64, "r": r}

        newG = stage_a2(stA2)
        stage_c(stC)
        stage_c(stage_b(stB))
        stage_c(stage_b(newG))

        # ---- normalize + store ---------------------------------------------
        gmax = const.tile([G16, 1], f32)
        nc.vector.tensor_reduce(gmax[:], tr_sb[:], axis=Axis.X, op=Alu.max)
        gmax_all = const.tile([G16, 1], f32)
        nc.gpsimd.partition_all_reduce(gmax_all[:], gmax[:], channels=G16,
                                       reduce_op=bass_isa.ReduceOp.max)
        denom = const.tile([G16, 1], f32)
        nc.vector.tensor_scalar_add(denom[:], gmax_all[:], 1e-8)
        rec = const.tile([G16, 1], f32)
        nc.vector.reciprocal(rec[:], denom[:])
        outv = const.tile([G16, NG * NBLK], f32)
        nc.scalar.activation(outv[:], tr_sb[:],
                             mybir.ActivationFunctionType.Copy, scale=rec[:])

        # invert the permutation: query at wrapped slot (q16, r*8+g) with
        # r = 8b+rb, g = 2gh+gl is point n = 1024b + 512gl + 32q16 + 4rb + gh
        # per-b DMA: n = 1024b + 512gl + 32q + (4rb+gh); partition q must be
        # the outermost SBUF dim, innermost runs are 32 contiguous elements
        ov = outv[:].rearrange("q (b gl rbgh) -> b q gl rbgh",
                               b=4, gl=2, rbgh=32)
        od = out_d.ap().rearrange("(b gl q rbgh) -> b q gl rbgh",
                                  b=4, gl=2, q=G16, rbgh=32)
        qs2 = (nc.sync, nc.scalar, nc.gpsimd, nc.sync)
        for b in range(4):
            qs2[b].dma_start(od[b], ov[b])

    nc.compile()
    return nc


_NC_CACHE = {}


def kernel(pcd, k):
    pcd = np.asarray(pcd)
    k = int(np.asarray(k))
    assert k == KNN, f"kernel hardcodes k={KNN}, got {k}"
    B, n, d = pcd.shape
    assert (n, d) == (N, 3), f"kernel hardcodes N={N}, got {(n, d)}"

    from concourse.bass_utils import run_bass_kernel_spmd

    if "nc" not in _NC_CACHE:
        _NC_CACHE["nc"] = build_nc()
    nc = _NC_CACHE["nc"]

    in_maps = [{"pcd": np.ascontiguousarray(pcd[b], dtype=np.float32)}
               for b in range(B)]
    res = run_bass_kernel_spmd(nc, in_maps, list(range(B)))
    out = np.stack([res.results[b]["out"] for b in range(B)], axis=0)
    return out.astype(np.float32, copy=False)


if __name__ == "__main__":
    x = np.random.randn(8, N, 3).astype(np.float32)
    y = kernel(x, 5)
    print(y.shape, y.dtype, y[:2, :4])


# revision 10
# speedup vs baseline: 1.0001x; 1.0001x over previous
"""Trainium2 Bass kernel for batched 3-D k-NN local-covariance trace.

Problem: pcd [B=8, N=4096, 3] -> per-point trace of the 3x3 covariance of its
k=5 nearest neighbors (self included), normalized by the per-batch max.

Sharding: data-parallel over batch — core b owns batch b (N=4096 points).

Per-core algorithm (all SBUF-resident after the initial load):
  * rank value r[i,j] = 2*x_i.y_j - |y_j|^2 computed as a bf16 matmul with an
    error-free hi/lo split (hi*hi + hi*lo + lo*hi), scaled by S=2^22, PLUS
    index-packing rows that make the PSUM value itself carry the candidate
    index (this removes the full-row FIND_INDEX8 pass entirely — the old
    bottleneck, ~170us of DVE time):
      row 11: per-query bias  bf16(OFF + S*(C - |x_i|^2))  -> top values land
              in (0, 2^24), independent of |x_i|
      row 12: +M, row 13: -M with M = 3*2^31.  fp32 PSUM accumulation is
              sequential in k, so +M rounds the running sum to a multiple of
              512 (ULP at 3*2^31 exponent) and -M restores it exactly
              (Sterbenz) — a free "round to 512" inside the matmul
      rows 14/15: jhi + jlo = the column's chunk-local index (0..511), which
              lands exactly in the 9 low bits freed by the rounding
    Net: psum = round512(S*(C' - d2)) + j, monotone in -d2 up to a 1.2e-4
    quantization (ties swap near-equal neighbors only; validated rel err
    ~4.5e-3 vs the fp32 reference).
  * selection per 128-query row block: one DVE MAX8 per 512-wide PSUM bank
    (top-8 per chunk, read directly from PSUM — no scalar copies), an 8x8
    merge MAX8 + MAX_INDEX on the [128,64] concat, then a tiny decode:
    j = int(packed) & 511, chunk = slot >> 3, global = chunk*512 + j.
  * setup avoids 4-byte-strided pcd loads: pcd is DMA'd once contiguously as
    [128, 96], hi/lo split runs elementwise on all 128 lanes, and the [*, N]
    matmul rows / gather table are produced by DVE 32x32 block transposes +
    contiguous flatten DMAs.  This stores points in a fixed permutation
    n' = 32p + j; every tensor uses the same permutation, so only the final
    output DMA needs to invert it.
  * neighbor coordinate gather via gpsimd indirect_copy (per-16-partition
    group, coords on partitions 16g..16g+2 of the table).
  * stable centered trace per query; components summed across partitions with
    a tiny matmul against a constant selection matrix E.
  * global max over the 4096 traces (gpsimd partition_all_reduce) -> scale by
    1/(max+1e-8) -> multi-queue DMA out.
"""

import numpy as np
from contextlib import ExitStack

N = 4096
KNN = 5
P = 128          # queries per row block
NBLK = N // P    # 32 row blocks
CH = 512         # candidate chunk (one fp32 PSUM bank)
NCH = N // CH    # 8 chunks
G16 = 16         # partitions per gpsimd core group
NG = P // G16    # 8 groups per row block
KA = 16          # augmented contraction rows (hi/lo split + pack rows)

SCL = float(2 ** 22)       # value scale (power of two: exact bf16 scaling)
CC = 1.5                   # clamp center: covers d5 up to CC + 2^23/SCL = 3.5
OFF = float(2 ** 23)       # positivity offset
MAGIC = float(3 * 2 ** 31)  # +MAGIC then -MAGIC rounds PSUM to multiples of 512


def build_nc():
    import concourse.bass as bass
    import concourse.tile as tile
    from concourse import bacc, mybir
    from concourse import bass_isa

    dt = mybir.dt
    f32 = dt.float32
    bf16 = dt.bfloat16
    Alu = mybir.AluOpType
    Axis = mybir.AxisListType

    nc = bacc.Bacc("TRN2", target_bir_lowering=False, debug=False)
    pcd_d = nc.dram_tensor("pcd", [N, 3], f32, kind="ExternalInput")
    out_d = nc.dram_tensor("out", [N], f32, kind="ExternalOutput")

    with tile.TileContext(nc) as tc, ExitStack() as ctx:
        const = ctx.enter_context(tc.tile_pool(name="const", bufs=1))
        stage = ctx.enter_context(tc.tile_pool(name="stage", bufs=1))
        small = ctx.enter_context(tc.tile_pool(name="small", bufs=4))
        psum = ctx.enter_context(tc.tile_pool(name="psum", bufs=7, space="PSUM"))
        psacc = ctx.enter_context(tc.tile_pool(name="psacc", bufs=1, space="PSUM"))

        # ---- one-time setup -------------------------------------------------
        # contiguous load: partition p holds points 32p..32p+31 (96 floats)
        pw = stage.tile([P, 3 * 32], f32)
        nc.sync.dma_start(pw[:], pcd_d.ap().rearrange("(p j) d -> p (j d)", p=P))

        # hi/lo split, elementwise in the wide layout (all 128 lanes)
        ph = stage.tile([P, 3 * 32], bf16)       # bf16(x)       (lhsT rows)
        nc.scalar.copy(ph[:], pw[:])
        p2h = stage.tile([P, 3 * 32], bf16)      # bf16(2*S*x)   (rhs rows)
        nc.scalar.mul(p2h[:], pw[:], 2.0 * SCL)
        phf = stage.tile([P, 3 * 32], f32)
        nc.vector.tensor_copy(phf[:], ph[:])
        plo = stage.tile([P, 3 * 32], f32)       # x - fp32(bf16(x)), exact
        nc.vector.tensor_sub(plo[:], pw[:], phf[:])
        plob = stage.tile([P, 3 * 32], bf16)
        nc.scalar.copy(plob[:], plo[:])
        p2lob = stage.tile([P, 3 * 32], bf16)    # bf16(2*S*lo)
        nc.scalar.mul(p2lob[:], plo[:], 2.0 * SCL)

        # -|y|^2 per point, then its own hi/lo split in the scaled domain
        s3p = stage.tile([P, 3 * 32], f32)
        nc.vector.tensor_mul(s3p[:], pw[:], pw[:])
        sqn = stage.tile([P, 32], f32)           # -|y|^2 (fp32)
        nc.vector.tensor_reduce(sqn[:], s3p[:].rearrange("p (j d) -> p j d", d=3),
                                axis=Axis.X, op=Alu.add, negate=True)
        snh = stage.tile([P, 32], bf16)          # bf16(S*sqn)
        nc.scalar.mul(snh[:], sqn[:], SCL)
        snhf = stage.tile([P, 32], f32)
        nc.vector.tensor_copy(snhf[:], snh[:])
        sqnS = stage.tile([P, 32], f32)
        nc.vector.tensor_scalar_mul(sqnS[:], sqn[:], SCL)
        snlo = stage.tile([P, 32], f32)
        nc.vector.tensor_sub(snlo[:], sqnS[:], snhf[:])
        snlob = stage.tile([P, 32], bf16)
        nc.scalar.copy(snlob[:], snlo[:])

        # per-query bias row: bf16(OFF + S*CC + S*sqn) = bf16(OFF + S*(CC-|x|^2))
        bqv = stage.tile([P, 32], bf16)
        nc.scalar.activation(bqv[:], sqn[:], mybir.ActivationFunctionType.Copy,
                             scale=SCL, bias=OFF + SCL * CC)

        # chunk-local index rows: column n' = 32p + j -> n' mod 512
        ji = stage.tile([P, 32], dt.int16)
        nc.gpsimd.iota(ji[:], [[1, 32]], channel_multiplier=32)
        jcl = stage.tile([P, 32], dt.int16)
        nc.vector.tensor_scalar(jcl[:], ji[:], 511, None, op0=Alu.bitwise_and)
        jhi16 = stage.tile([P, 32], dt.int16)
        nc.vector.tensor_scalar(jhi16[:], jcl[:], 504, None, op0=Alu.bitwise_and)
        jlo16 = stage.tile([P, 32], dt.int16)
        nc.vector.tensor_scalar(jlo16[:], jcl[:], 7, None, op0=Alu.bitwise_and)
        jhib = stage.tile([P, 32], bf16)
        nc.vector.tensor_copy(jhib[:], jhi16[:])
        jlob = stage.tile([P, 32], bf16)
        nc.vector.tensor_copy(jlob[:], jlo16[:])

        # 32x32 block transposes; flattening (partition-major) then yields the
        # permuted point order n' shared by every [*, N] tensor below.
        def coord_view(t, d):
            return t[:].rearrange("p (j d) -> p d j", d=3)[:, d, :]

        tbl = const.tile([P, N], f32)        # gather table: coords on p%16<3
        xl = const.tile([KA, N], bf16)       # lhsT rows
        xr = const.tile([KA, N], bf16)       # rhs rows

        tp = [stage.tile([P, 32], f32, name=f"tp{d}") for d in range(3)]
        th = [stage.tile([P, 32], bf16, name=f"th{d}") for d in range(3)]
        t2h = [stage.tile([P, 32], bf16, name=f"t2h{d}") for d in range(3)]
        tlo = [stage.tile([P, 32], bf16, name=f"tlo{d}") for d in range(3)]
        t2lo = [stage.tile([P, 32], bf16, name=f"t2lo{d}") for d in range(3)]
        tsh = stage.tile([P, 32], bf16)
        tslo = stage.tile([P, 32], bf16)
        tbq = stage.tile([P, 32], bf16)
        for d in range(3):
            nc.vector.transpose(tp[d][:], coord_view(pw, d))
            nc.vector.transpose(th[d][:], coord_view(ph, d))
            nc.vector.transpose(t2h[d][:], coord_view(p2h, d))
            nc.vector.transpose(tlo[d][:], coord_view(plob, d))
            nc.vector.transpose(t2lo[d][:], coord_view(p2lob, d))
        nc.vector.transpose(tsh[:], snh[:])
        nc.vector.transpose(tslo[:], snlob[:])
        nc.vector.transpose(tbq[:], bqv[:])

        # constant rows (ones / +-MAGIC): gpsimd memsets must start at a
        # group-aligned partition, so memset the whole lhsT tile to ones
        # (value/bias rows are overwritten by the flatten DMAs below) and
        # stage the xr constant rows in partition-0 tiles.
        nc.gpsimd.memset(xl[:, :], 1.0)
        onesr = stage.tile([1, N], bf16)
        nc.gpsimd.memset(onesr[:], 1.0)
        magr = stage.tile([1, N], bf16)
        nc.gpsimd.memset(magr[:], MAGIC)
        nmagr = stage.tile([1, N], bf16)
        nc.gpsimd.memset(nmagr[:], -MAGIC)
        nc.sync.dma_start(xr[11:12, :], onesr[:])
        nc.scalar.dma_start(xr[12:13, :], magr[:])
        nc.gpsimd.dma_start(xr[13:14, :], nmagr[:])

        # flatten DMAs (contiguous 128B/64B runs) into the row tiles;
        # xr/xl first so the main loop's matmuls can start ASAP (DMA queues
        # drain in issue order), the gather table after
        qs = (nc.sync, nc.scalar, nc.gpsimd)
        for d in range(3):
            qs[d].dma_start(xr[d:d + 1, :], t2h[d][:])
            qs[(d + 1) % 3].dma_start(xr[3 + d:4 + d, :], t2lo[d][:])
            qs[(d + 2) % 3].dma_start(xl[d:d + 1, :], th[d][:])
            qs[d].dma_start(xl[6 + d:7 + d, :], tlo[d][:])
        for d in range(3):
            qs[(d + 1) % 3].dma_start(xl[3 + d:4 + d, :], th[d][:])
            qs[(d + 2) % 3].dma_start(xr[6 + d:7 + d, :], t2h[d][:])
        nc.sync.dma_start(xr[9:10, :], tsh[:])
        nc.scalar.dma_start(xr[10:11, :], tslo[:])
        nc.gpsimd.dma_start(xl[11:12, :], tbq[:])
        nc.sync.dma_start(xr[14:15, :], jhib[:])
        nc.scalar.dma_start(xr[15:16, :], jlob[:])
        nc.gpsimd.memset(tbl[:], 0.0)
        for d in range(3):
            qs[d].dma_start(tbl[d:d + 1, :], tp[d][:])

        # replicate coords to every 16-partition group of tbl
        engs = (nc.sync, nc.scalar, nc.gpsimd, nc.sync,
                nc.scalar, nc.gpsimd, nc.sync)
        for g in range(1, NG):
            engs[g - 1].dma_start(tbl[G16 * g:G16 * g + 3, :], tbl[0:3, :])

        # E[p, g] = 1 iff p//16 == g and p%16 < 3  (component-sum selector)
        ones3 = const.tile([3, 1], f32)
        nc.vector.memset(ones3[:], 1.0)
        esel = const.tile([P, NG], f32)
        nc.vector.memset(esel[:], 0.0)
        for j in range(NG):
            g = 2 * (j & 3) + (j >> 2)
            nc.sync.dma_start(esel[G16 * g:G16 * g + 3, j:j + 1], ones3[:])

        warm = stage.tile([G16, 1], f32)
        nc.vector.memset(warm[:], 0.0)
        warm2 = stage.tile([G16, 1], f32)
        nc.gpsimd.partition_all_reduce(warm2[:], warm[:], channels=G16,
                                       reduce_op=bass_isa.ReduceOp.max)

        tr_sb = const.tile([G16, NG * NBLK], f32)
        # free layout of tr_sb: f = 64b + 32gl + 4rb + gh for row block
        # r = 8b + rb and group g = 2gh + gl  ->  DRAM block b is contiguous
        tr_view = tr_sb[:].rearrange("q (b gl rb gh) -> q b gl rb gh",
                                     b=4, gl=2, rb=8, gh=4)

        # ---- main loop over row blocks (4-stage software pipeline) ---------
        # stage A1(r):  matmuls -> per-chunk MAX8 into v64
        # stage A2(r-1): merge/max_index -> decode -> gather issue
        # stage B(r-2): ssum -> mean -> centered squares (gpsimd)
        # stage C(r-3): tt reduce -> component-sum matmul -> tr store
        # Tail stages run behind so the strict-FIFO DVE queue never
        # head-of-line blocks the next block's MAX8 scans on cross-engine
        # latency (gather on gpsimd, mean on scalar).
        def stage_a2(st):
            r, v64 = st["r"], st["v64"]
            vm8 = small.tile([P, 8], f32, tag="vm8")
            nc.vector.max(vm8[:], v64[:])
            slot8 = small.tile([P, 8], dt.uint16, tag="slot8")
            nc.vector.max_index(slot8[:], vm8[:], v64[:])
            # decode: j = int(packed) & 511 ; chunk = slot>>3 ; g = chunk*512+j
            pki = small.tile([P, 8], dt.int32, tag="pki")
            nc.vector.tensor_copy(pki[:], vm8[:])
            jt = small.tile([P, 8], dt.int32, tag="jt")
            nc.vector.tensor_scalar(jt[:], pki[:], 511, None, op0=Alu.bitwise_and)
            jt16 = small.tile([P, 8], dt.uint16, tag="jt16")
            nc.vector.tensor_copy(jt16[:], jt[:])
            cb = small.tile([P, 8], dt.uint16, tag="cb")
            nc.vector.tensor_scalar(cb[:], slot8[:], 3, 9,
                                    op0=Alu.logical_shift_right,
                                    op1=Alu.logical_shift_left)
            gidx = small.tile([P, 8], dt.uint16, tag="gidx")
            nc.vector.tensor_add(gidx[:], cb[:], jt16[:])
            # gather: group g gathers, for its 16 queries, slot-major:
            # gath[p, s*16+q16] = tbl[p, gidx[16*(p//16)+q16, s]]
            gath = small.tile([P, KNN * G16], f32, tag="gath")
            nc.gpsimd.indirect_copy(gath[:], tbl[:], gidx[:, :KNN], True)
            return {"gath": gath, "r": r}

        def stage_b(st):
            gv = st["gath"][:].rearrange("p (s q) -> p q s", s=KNN, q=G16)
            ssum = small.tile([P, G16], f32, tag="ssum")
            nc.vector.tensor_reduce(ssum[:], gv, axis=Axis.X, op=Alu.add)
            mean = small.tile([P, G16], f32, tag="mean")
            nc.scalar.mul(mean[:], ssum[:], 1.0 / KNN)
            cent = small.tile([P, G16, KNN], f32, tag="cent")
            nc.gpsimd.tensor_sub(cent[:], gv,
                                 mean[:].unsqueeze(2).broadcast_to([P, G16, KNN]))
            nc.gpsimd.tensor_mul(cent[:], cent[:], cent[:])
            return {"cent": cent, "r": st["r"]}

        def stage_c(st):
            r = st["r"]
            tt = small.tile([P, G16], f32, tag="tt")
            nc.vector.tensor_reduce(tt[:], st["cent"][:], axis=Axis.X, op=Alu.add)
            ps_tr = psacc.tile([G16, NG], f32, tag="tr")
            nc.tensor.matmul(ps_tr[:], tt[:], esel[:], start=True, stop=True)
            nc.scalar.copy(tr_view[:, r // 8, :, r % 8, :],
                           ps_tr[:].rearrange("q (gl gh) -> q gl gh", gl=2))

        stA2 = stB = stC = None
        for r in range(NBLK):
            lhsT = xl[:, r * P:(r + 1) * P]
            v64 = small.tile([P, NCH * 8], f32, tag="v64")
            for c in range(NCH):
                sl = slice(c * CH, (c + 1) * CH)
                ps = psum.tile([P, CH], f32, tag="mm")
                nc.tensor.matmul(ps[:], lhsT, xr[:, sl], start=True, stop=True)
                # top-8 of this chunk, straight from the PSUM bank
                nc.vector.max(v64[:, 8 * c:8 * c + 8], ps[:])

            newG = stage_a2(stA2) if stA2 else None
            if stC:
                stage_c(stC)
            stC = stage_b(stB) if stB else None
            stB = newG
            stA2 = {"v64": v64, "r": r}

        newG = stage_a2(stA2)
        stage_c(stC)
        stage_c(stage_b(stB))
        stage_c(stage_b(newG))

        # ---- normalize + store ---------------------------------------------
        gmax = const.tile([G16, 1], f32)
        nc.vector.tensor_reduce(gmax[:], tr_sb[:], axis=Axis.X, op=Alu.max)
        gmax_all = const.tile([G16, 1], f32)
        nc.gpsimd.partition_all_reduce(gmax_all[:], gmax[:], channels=G16,
                                       reduce_op=bass_isa.ReduceOp.max)
        denom = const.tile([G16, 1], f32)
        nc.vector.tensor_scalar_add(denom[:], gmax_all[:], 1e-8)
        rec = const.tile([G16, 1], f32)
        nc.vector.reciprocal(rec[:], denom[:])
        outv = const.tile([G16, NG * NBLK], f32)
        nc.scalar.activation(outv[:], tr_sb[:],
                             mybir.ActivationFunctionType.Copy, scale=rec[:])

        # invert the permutation: query at wrapped slot (q16, r*8+g) with
        # r = 8b+rb, g = 2gh+gl is point n = 1024b + 512gl + 32q16 + 4rb + gh
        # per-b DMA: n = 1024b + 512gl + 32q + (4rb+gh); partition q must be
        # the outermost SBUF dim, innermost runs are 32 contiguous elements
        ov = outv[:].rearrange("q (b gl rbgh) -> b q gl rbgh",
                               b=4, gl=2, rbgh=32)
        od = out_d.ap().rearrange("(b gl q rbgh) -> b q gl rbgh",
                                  b=4, gl=2, q=G16, rbgh=32)
        qs2 = (nc.sync, nc.scalar, nc.gpsimd, nc.sync)
        for b in range(4):
            qs2[b].dma_start(od[b], ov[b])

    nc.compile()
    return nc


_NC_CACHE = {}


def kernel(pcd, k):
    pcd = np.asarray(pcd)
    k = int(np.asarray(k))
    assert k == KNN, f"kernel hardcodes k={KNN}, got {k}"
    B, n, d = pcd.shape
    assert (n, d) == (N, 3), f"kernel hardcodes N={N}, got {(n, d)}"

    from concourse.bass_utils import run_bass_kernel_spmd

    if "nc" not in _NC_CACHE:
        _NC_CACHE["nc"] = build_nc()
    nc = _NC_CACHE["nc"]

    in_maps = [{"pcd": np.ascontiguousarray(pcd[b], dtype=np.float32)}
               for b in range(B)]
    res = run_bass_kernel_spmd(nc, in_maps, list(range(B)))
    out = np.stack([res.results[b]["out"] for b in range(B)], axis=0)
    return out.astype(np.float32, copy=False)


if __name__ == "__main__":
    x = np.random.randn(8, N, 3).astype(np.float32)
    y = kernel(x, 5)
    print(y.shape, y.dtype, y[:2, :4])


# revision 12
# speedup vs baseline: 1.0231x; 1.0230x over previous
"""Trainium2 Bass kernel for batched 3-D k-NN local-covariance trace.

Problem: pcd [B=8, N=4096, 3] -> per-point trace of the 3x3 covariance of its
k=5 nearest neighbors (self included), normalized by the per-batch max.

Sharding: data-parallel over batch — core b owns batch b (N=4096 points).

Per-core algorithm (all SBUF-resident after the initial load):
  * rank value r[i,j] = 2*x_i.y_j - |y_j|^2 computed as a bf16 matmul with an
    error-free hi/lo split (hi*hi + hi*lo + lo*hi), scaled by S=2^22, PLUS
    index-packing rows that make the PSUM value itself carry the candidate
    index (this removes the full-row FIND_INDEX8 pass entirely — the old
    bottleneck, ~170us of DVE time):
      row 11: per-query bias  bf16(OFF + S*(C - |x_i|^2))  -> top values land
              in (0, 2^24), independent of |x_i|
      row 12: +M, row 13: -M with M = 3*2^31.  fp32 PSUM accumulation is
              sequential in k, so +M rounds the running sum to a multiple of
              512 (ULP at 3*2^31 exponent) and -M restores it exactly
              (Sterbenz) — a free "round to 512" inside the matmul
      rows 14/15: jhi + jlo = the column's chunk-local index (0..511), which
              lands exactly in the 9 low bits freed by the rounding
    Net: psum = round512(S*(C' - d2)) + j, monotone in -d2 up to a 1.2e-4
    quantization (ties swap near-equal neighbors only; validated rel err
    ~4.5e-3 vs the fp32 reference).
  * selection per 128-query row block: one DVE MAX8 per 512-wide PSUM bank
    (top-8 per chunk, read directly from PSUM — no scalar copies), an 8x8
    merge MAX8 + MAX_INDEX on the [128,64] concat, then a tiny decode:
    j = int(packed) & 511, chunk = slot >> 3, global = chunk*512 + j.
  * setup avoids 4-byte-strided pcd loads: pcd is DMA'd once contiguously as
    [128, 96], hi/lo split runs elementwise on all 128 lanes, and the [*, N]
    matmul rows / gather table are produced by DVE 32x32 block transposes +
    contiguous flatten DMAs.  This stores points in a fixed permutation
    n' = 32p + j; every tensor uses the same permutation, so only the final
    output DMA needs to invert it.
  * neighbor coordinate gather via gpsimd indirect_copy (per-16-partition
    group, coords on partitions 16g..16g+2 of the table).
  * stable centered trace per query; components summed across partitions with
    a tiny matmul against a constant selection matrix E.
  * global max over the 4096 traces (gpsimd partition_all_reduce) -> scale by
    1/(max+1e-8) -> multi-queue DMA out.
"""

import numpy as np
from contextlib import ExitStack

N = 4096
KNN = 5
P = 128          # queries per row block
NBLK = N // P    # 32 row blocks
CH = 512         # candidate chunk (one fp32 PSUM bank)
NCH = N // CH    # 8 chunks
G16 = 16         # partitions per gpsimd core group
NG = P // G16    # 8 groups per row block
KA = 16          # augmented contraction rows (hi/lo split + pack rows)

SCL = float(2 ** 22)       # value scale (power of two: exact bf16 scaling)
CC = 1.5                   # clamp center: covers d5 up to CC + 2^23/SCL = 3.5
OFF = float(2 ** 23)       # positivity offset
MAGIC = float(3 * 2 ** 31)  # +MAGIC then -MAGIC rounds PSUM to multiples of 512


def build_nc():
    import concourse.bass as bass
    import concourse.tile as tile
    from concourse import bacc, mybir
    from concourse import bass_isa

    dt = mybir.dt
    f32 = dt.float32
    bf16 = dt.bfloat16
    Alu = mybir.AluOpType
    Axis = mybir.AxisListType

    nc = bacc.Bacc("TRN2", target_bir_lowering=False, debug=False)
    pcd_d = nc.dram_tensor("pcd", [N, 3], f32, kind="ExternalInput")
    out_d = nc.dram_tensor("out", [N], f32, kind="ExternalOutput")

    with tile.TileContext(nc) as tc, ExitStack() as ctx:
        const = ctx.enter_context(tc.tile_pool(name="const", bufs=1))
        stage = ctx.enter_context(tc.tile_pool(name="stage", bufs=1))
        small = ctx.enter_context(tc.tile_pool(name="small", bufs=4))
        psum = ctx.enter_context(tc.tile_pool(name="psum", bufs=7, space="PSUM"))
        psacc = ctx.enter_context(tc.tile_pool(name="psacc", bufs=1, space="PSUM"))

        # ---- one-time setup -------------------------------------------------
        # contiguous load: partition p holds points 32p..32p+31 (96 floats)
        pw = stage.tile([P, 3 * 32], f32)
        nc.sync.dma_start(pw[:], pcd_d.ap().rearrange("(p j) d -> p (j d)", p=P))

        # constant staging in the cheap wide layout (tiny memsets, issued
        # first so the gpsimd FIFO isn't clogged ahead of the row DMAs)
        ones32 = stage.tile([P, 32], bf16)
        nc.gpsimd.memset(ones32[:], 1.0)
        mag32 = stage.tile([P, 32], bf16)
        nc.gpsimd.memset(mag32[:], MAGIC)
        nmag32 = stage.tile([P, 32], bf16)
        nc.gpsimd.memset(nmag32[:], -MAGIC)

        # hi/lo split, elementwise in the wide layout (all 128 lanes)
        ph = stage.tile([P, 3 * 32], bf16)       # bf16(x)       (lhsT rows)
        nc.scalar.copy(ph[:], pw[:])
        p2h = stage.tile([P, 3 * 32], bf16)      # bf16(2*S*x)   (rhs rows)
        nc.scalar.mul(p2h[:], pw[:], 2.0 * SCL)
        phf = stage.tile([P, 3 * 32], f32)
        nc.vector.tensor_copy(phf[:], ph[:])
        plo = stage.tile([P, 3 * 32], f32)       # x - fp32(bf16(x)), exact
        nc.vector.tensor_sub(plo[:], pw[:], phf[:])
        plob = stage.tile([P, 3 * 32], bf16)
        nc.scalar.copy(plob[:], plo[:])
        p2lob = stage.tile([P, 3 * 32], bf16)    # bf16(2*S*lo)
        nc.scalar.mul(p2lob[:], plo[:], 2.0 * SCL)

        # -|y|^2 per point, then its own hi/lo split in the scaled domain
        s3p = stage.tile([P, 3 * 32], f32)
        nc.vector.tensor_mul(s3p[:], pw[:], pw[:])
        sqn = stage.tile([P, 32], f32)           # -|y|^2 (fp32)
        nc.vector.tensor_reduce(sqn[:], s3p[:].rearrange("p (j d) -> p j d", d=3),
                                axis=Axis.X, op=Alu.add, negate=True)
        snh = stage.tile([P, 32], bf16)          # bf16(S*sqn)
        nc.scalar.mul(snh[:], sqn[:], SCL)
        snhf = stage.tile([P, 32], f32)
        nc.vector.tensor_copy(snhf[:], snh[:])
        sqnS = stage.tile([P, 32], f32)
        nc.vector.tensor_scalar_mul(sqnS[:], sqn[:], SCL)
        snlo = stage.tile([P, 32], f32)
        nc.vector.tensor_sub(snlo[:], sqnS[:], snhf[:])
        snlob = stage.tile([P, 32], bf16)
        nc.scalar.copy(snlob[:], snlo[:])

        # per-query bias row: bf16(OFF + S*CC + S*sqn) = bf16(OFF + S*(CC-|x|^2))
        bqv = stage.tile([P, 32], bf16)
        nc.scalar.activation(bqv[:], sqn[:], mybir.ActivationFunctionType.Copy,
                             scale=SCL, bias=OFF + SCL * CC)

        # chunk-local index rows: column n' = 32p + j -> n' mod 512
        ji = stage.tile([P, 32], dt.int16)
        nc.gpsimd.iota(ji[:], [[1, 32]], channel_multiplier=32)
        jcl = stage.tile([P, 32], dt.int16)
        nc.vector.tensor_scalar(jcl[:], ji[:], 511, None, op0=Alu.bitwise_and)
        jhi16 = stage.tile([P, 32], dt.int16)
        nc.vector.tensor_scalar(jhi16[:], jcl[:], 504, None, op0=Alu.bitwise_and)
        jlo16 = stage.tile([P, 32], dt.int16)
        nc.vector.tensor_scalar(jlo16[:], jcl[:], 7, None, op0=Alu.bitwise_and)
        jhib = stage.tile([P, 32], bf16)
        nc.vector.tensor_copy(jhib[:], jhi16[:])
        jlob = stage.tile([P, 32], bf16)
        nc.vector.tensor_copy(jlob[:], jlo16[:])

        # 32x32 block transposes; flattening (partition-major) then yields the
        # permuted point order n' shared by every [*, N] tensor below.
        def coord_view(t, d):
            return t[:].rearrange("p (j d) -> p d j", d=3)[:, d, :]

        tbl = const.tile([P, N], f32)        # gather table: coords on p%16<3
        xl = const.tile([KA, N], bf16)       # lhsT rows
        xr = const.tile([KA, N], bf16)       # rhs rows

        tp = [stage.tile([P, 32], f32, name=f"tp{d}") for d in range(3)]
        th = [stage.tile([P, 32], bf16, name=f"th{d}") for d in range(3)]
        t2h = [stage.tile([P, 32], bf16, name=f"t2h{d}") for d in range(3)]
        tlo = [stage.tile([P, 32], bf16, name=f"tlo{d}") for d in range(3)]
        t2lo = [stage.tile([P, 32], bf16, name=f"t2lo{d}") for d in range(3)]
        tsh = stage.tile([P, 32], bf16)
        tslo = stage.tile([P, 32], bf16)
        tbq = stage.tile([P, 32], bf16)
        for d in range(3):
            nc.vector.transpose(tp[d][:], coord_view(pw, d))
            nc.vector.transpose(th[d][:], coord_view(ph, d))
            nc.vector.transpose(t2h[d][:], coord_view(p2h, d))
            nc.vector.transpose(tlo[d][:], coord_view(plob, d))
            nc.vector.transpose(t2lo[d][:], coord_view(p2lob, d))
        nc.vector.transpose(tsh[:], snh[:])
        nc.vector.transpose(tslo[:], snlob[:])
        nc.vector.transpose(tbq[:], bqv[:])

        # constant rows (ones / +-MAGIC) via flatten DMAs from the small
        # staging tiles (a [P,32] flatten is the same contiguous-run pattern
        # as the transposed value rows)
        nc.sync.dma_start(xr[11:12, :], ones32[:])
        nc.scalar.dma_start(xr[12:13, :], mag32[:])
        nc.gpsimd.dma_start(xr[13:14, :], nmag32[:])
        nc.sync.dma_start(xl[9:10, :], ones32[:])
        nc.scalar.dma_start(xl[10:11, :], ones32[:])
        nc.gpsimd.dma_start(xl[12:13, :], ones32[:])
        nc.sync.dma_start(xl[13:14, :], ones32[:])
        nc.scalar.dma_start(xl[14:15, :], ones32[:])
        nc.gpsimd.dma_start(xl[15:16, :], ones32[:])

        # flatten DMAs (contiguous 128B/64B runs) into the row tiles;
        # xr/xl first so the main loop's matmuls can start ASAP (DMA queues
        # drain in issue order), the gather table after
        qs = (nc.sync, nc.scalar, nc.gpsimd)
        for d in range(3):
            qs[d].dma_start(xr[d:d + 1, :], t2h[d][:])
            qs[(d + 1) % 3].dma_start(xr[3 + d:4 + d, :], t2lo[d][:])
            qs[(d + 2) % 3].dma_start(xl[d:d + 1, :], th[d][:])
            qs[d].dma_start(xl[6 + d:7 + d, :], tlo[d][:])
        for d in range(3):
            qs[(d + 1) % 3].dma_start(xl[3 + d:4 + d, :], th[d][:])
            qs[(d + 2) % 3].dma_start(xr[6 + d:7 + d, :], t2h[d][:])
        nc.sync.dma_start(xr[9:10, :], tsh[:])
        nc.scalar.dma_start(xr[10:11, :], tslo[:])
        nc.gpsimd.dma_start(xl[11:12, :], tbq[:])
        nc.sync.dma_start(xr[14:15, :], jhib[:])
        nc.scalar.dma_start(xr[15:16, :], jlob[:])
        nc.gpsimd.memset(tbl[:], 0.0)
        for d in range(3):
            qs[d].dma_start(tbl[d:d + 1, :], tp[d][:])

        # replicate coords to every 16-partition group of tbl
        engs = (nc.sync, nc.scalar, nc.gpsimd, nc.sync,
                nc.scalar, nc.gpsimd, nc.sync)
        for g in range(1, NG):
            engs[g - 1].dma_start(tbl[G16 * g:G16 * g + 3, :], tbl[0:3, :])

        # E[p, g] = 1 iff p//16 == g and p%16 < 3  (component-sum selector)
        ones3 = const.tile([3, 1], f32)
        nc.vector.memset(ones3[:], 1.0)
        esel = const.tile([P, NG], f32)
        nc.vector.memset(esel[:], 0.0)
        for j in range(NG):
            g = 2 * (j & 3) + (j >> 2)
            nc.sync.dma_start(esel[G16 * g:G16 * g + 3, j:j + 1], ones3[:])

        warm = stage.tile([G16, 1], f32)
        nc.vector.memset(warm[:], 0.0)
        warm2 = stage.tile([G16, 1], f32)
        nc.gpsimd.partition_all_reduce(warm2[:], warm[:], channels=G16,
                                       reduce_op=bass_isa.ReduceOp.max)

        tr_sb = const.tile([G16, NG * NBLK], f32)
        # free layout of tr_sb: f = 64b + 32gl + 4rb + gh for row block
        # r = 8b + rb and group g = 2gh + gl  ->  DRAM block b is contiguous
        tr_view = tr_sb[:].rearrange("q (b gl rb gh) -> q b gl rb gh",
                                     b=4, gl=2, rb=8, gh=4)

        # ---- main loop over row blocks (4-stage software pipeline) ---------
        # stage A1(r):  matmuls -> per-chunk MAX8 into v64
        # stage A2(r-1): merge/max_index -> decode -> gather issue
        # stage B(r-2): ssum -> mean -> centered squares (gpsimd)
        # stage C(r-3): tt reduce -> component-sum matmul -> tr store
        # Tail stages run behind so the strict-FIFO DVE queue never
        # head-of-line blocks the next block's MAX8 scans on cross-engine
        # latency (gather on gpsimd, mean on scalar).
        def stage_a2(st):
            r, v64 = st["r"], st["v64"]
            vm8 = small.tile([P, 8], f32, tag="vm8")
            nc.vector.max(vm8[:], v64[:])
            slot8 = small.tile([P, 8], dt.uint16, tag="slot8")
            nc.vector.max_index(slot8[:], vm8[:], v64[:])
            # decode: j = int(packed) & 511 ; chunk = slot>>3 ; g = chunk*512+j
            pki = small.tile([P, 8], dt.int32, tag="pki")
            nc.vector.tensor_copy(pki[:], vm8[:])
            jt = small.tile([P, 8], dt.int32, tag="jt")
            nc.vector.tensor_scalar(jt[:], pki[:], 511, None, op0=Alu.bitwise_and)
            jt16 = small.tile([P, 8], dt.uint16, tag="jt16")
            nc.vector.tensor_copy(jt16[:], jt[:])
            cb = small.tile([P, 8], dt.uint16, tag="cb")
            nc.vector.tensor_scalar(cb[:], slot8[:], 3, 9,
                                    op0=Alu.logical_shift_right,
                                    op1=Alu.logical_shift_left)
            gidx = small.tile([P, 8], dt.uint16, tag="gidx")
            nc.vector.tensor_add(gidx[:], cb[:], jt16[:])
            # gather: group g gathers, for its 16 queries, slot-major:
            # gath[p, s*16+q16] = tbl[p, gidx[16*(p//16)+q16, s]]
            gath = small.tile([P, KNN * G16], f32, tag="gath")
            nc.gpsimd.indirect_copy(gath[:], tbl[:], gidx[:, :KNN], True)
            return {"gath": gath, "r": r}

        def stage_b(st):
            gv = st["gath"][:].rearrange("p (s q) -> p q s", s=KNN, q=G16)
            ssum = small.tile([P, G16], f32, tag="ssum")
            nc.vector.tensor_reduce(ssum[:], gv, axis=Axis.X, op=Alu.add)
            mean = small.tile([P, G16], f32, tag="mean")
            nc.scalar.mul(mean[:], ssum[:], 1.0 / KNN)
            cent = small.tile([P, G16, KNN], f32, tag="cent")
            nc.gpsimd.tensor_sub(cent[:], gv,
                                 mean[:].unsqueeze(2).broadcast_to([P, G16, KNN]))
            nc.gpsimd.tensor_mul(cent[:], cent[:], cent[:])
            return {"cent": cent, "r": st["r"]}

        def stage_c(st):
            r = st["r"]
            tt = small.tile([P, G16], f32, tag="tt")
            nc.vector.tensor_reduce(tt[:], st["cent"][:], axis=Axis.X, op=Alu.add)
            ps_tr = psacc.tile([G16, NG], f32, tag="tr")
            nc.tensor.matmul(ps_tr[:], tt[:], esel[:], start=True, stop=True)
            nc.scalar.copy(tr_view[:, r // 8, :, r % 8, :],
                           ps_tr[:].rearrange("q (gl gh) -> q gl gh", gl=2))

        stA2 = stB = stC = None
        for r in range(NBLK):
            lhsT = xl[:, r * P:(r + 1) * P]
            v64 = small.tile([P, NCH * 8], f32, tag="v64")
            for c in range(NCH):
                sl = slice(c * CH, (c + 1) * CH)
                ps = psum.tile([P, CH], f32, tag="mm")
                nc.tensor.matmul(ps[:], lhsT, xr[:, sl], start=True, stop=True)
                # top-8 of this chunk, straight from the PSUM bank
                nc.vector.max(v64[:, 8 * c:8 * c + 8], ps[:])

            newG = stage_a2(stA2) if stA2 else None
            if stC:
                stage_c(stC)
            stC = stage_b(stB) if stB else None
            stB = newG
            stA2 = {"v64": v64, "r": r}

        newG = stage_a2(stA2)
        stage_c(stC)
        stage_c(stage_b(stB))
        stage_c(stage_b(newG))

        # ---- normalize + store ---------------------------------------------
        gmax = const.tile([G16, 1], f32)
        nc.vector.tensor_reduce(gmax[:], tr_sb[:], axis=Axis.X, op=Alu.max)
        gmax_all = const.tile([G16, 1], f32)
        nc.gpsimd.partition_all_reduce(gmax_all[:], gmax[:], channels=G16,
                                       reduce_op=bass_isa.ReduceOp.max)
        denom = const.tile([G16, 1], f32)
        nc.vector.tensor_scalar_add(denom[:], gmax_all[:], 1e-8)
        rec = const.tile([G16, 1], f32)
        nc.vector.reciprocal(rec[:], denom[:])
        outv = const.tile([G16, NG * NBLK], f32)
        nc.scalar.activation(outv[:], tr_sb[:],
                             mybir.ActivationFunctionType.Copy, scale=rec[:])

        # invert the permutation: query at wrapped slot (q16, r*8+g) with
        # r = 8b+rb, g = 2gh+gl is point n = 1024b + 512gl + 32q16 + 4rb + gh
        # per-b DMA: n = 1024b + 512gl + 32q + (4rb+gh); partition q must be
        # the outermost SBUF dim, innermost runs are 32 contiguous elements
        ov = outv[:].rearrange("q (b gl rbgh) -> b q gl rbgh",
                               b=4, gl=2, rbgh=32)
        od = out_d.ap().rearrange("(b gl q rbgh) -> b q gl rbgh",
                                  b=4, gl=2, q=G16, rbgh=32)
        qs2 = (nc.sync, nc.scalar, nc.gpsimd, nc.sync)
        for b in range(4):
            qs2[b].dma_start(od[b], ov[b])

    nc.compile()
    return nc


_NC_CACHE = {}


def kernel(pcd, k):
    pcd = np.asarray(pcd)
    k = int(np.asarray(k))
    assert k == KNN, f"kernel hardcodes k={KNN}, got {k}"
    B, n, d = pcd.shape
    assert (n, d) == (N, 3), f"kernel hardcodes N={N}, got {(n, d)}"

    from concourse.bass_utils import run_bass_kernel_spmd

    if "nc" not in _NC_CACHE:
        _NC_CACHE["nc"] = build_nc()
    nc = _NC_CACHE["nc"]

    in_maps = [{"pcd": np.ascontiguousarray(pcd[b], dtype=np.float32)}
               for b in range(B)]
    res = run_bass_kernel_spmd(nc, in_maps, list(range(B)))
    out = np.stack([res.results[b]["out"] for b in range(B)], axis=0)
    return out.astype(np.float32, copy=False)


if __name__ == "__main__":
    x = np.random.randn(8, N, 3).astype(np.float32)
    y = kernel(x, 5)
    print(y.shape, y.dtype, y[:2, :4])
